# revision 60
# baseline (speedup 1.0000x reference)
"""BiMamba (bidirectional Mamba block) Trainium2 kernel — fused fp16 pipeline.

Contract: kernel(**inputs) takes the full (unsharded) numpy inputs of the
reference and returns the full (2, 4096, 1024) float32 output.

Sharding: 8 cores = 2 batches x 4 channel-groups of 512 d_inner channels.
Each core runs both scan directions for its channel slice; the x_dbl
reduction over d_inner is an on-chip AllReduce within each batch's 4-core
group (done per chunk so the forward scan fuses into pass A); the host sums
the four partial out-projections per batch.

Key algebraic facts used:
  * xz for the reverse direction is the L-flip of the forward xz, so the
    input projection is computed once.
  * (y_f + flip(y_r)) @ W_out.T == out_f + flip(out_r), so one output
    projection suffices.

Structure:
  PASS A (per forward chunk): in_proj -> depthwise conv (both dirs) ->
    per-chunk x_dbl AllReduce -> forward-direction selective scan fused in
    (dt, B/C broadcasts, 16-state scan, PSUM-matmul state reduction,
    gating); gated forward output stays in SBUF.  Reverse-conv output is
    spilled to HBM in forward time (contiguous).
  PASS B (per flipped chunk): reverse-direction scan (reading the spill
    with reversed in-SBUF access patterns), combine with the stored forward
    output, output projection.

Performance notes:
  * All elementwise tensors fp16 (DVE 2x mode; scan state stays fp32).
  * y = sum_s hs*C_s accumulates in PSUM via identity-weight matmuls (PE).
  * B_s/C_s broadcasts: matmul pair into one 2-bank PSUM tile, one wide
    copy to SBUF fp16 (split between Act and DVE).
  * bt/cm multiplies split between DVE and Pool (tunable PB/PC).
"""

import os
import sys

import numpy as np

sys.path.insert(0, "/opt/trn_rl_repo")

B, L, DM, DI, DS, DR, DC = 2, 4096, 1024, 2048, 16, 64, 4
CH = 512          # d_inner channels per core
NCH = CH // 128   # channel tiles per core
T1 = 512          # chunk size (shared by pass A and pass B)
NC1 = L // T1
T2 = 512
NC2 = L // T2

_COMPILED = [None]
# Pool-offload tuning: of the 16 (s) slots per m, how many bt / cm multiplies
# go to Pool (gpsimd); remainder run on DVE.
_PB = int(os.environ.get("PB", "6"))    # bt ops to Pool per 16
_PC = int(os.environ.get("PC", "7"))    # cm ops to Pool per 16
_BCD = int(os.environ.get("BCD", "0"))  # of 16 bct copies, how many on DVE
_PBR = int(os.environ.get("PBR", "6"))   # pass-B bt ops to Pool per 16
_PCR = int(os.environ.get("PCR", "8"))   # pass-B cm ops to Pool per 16


def _split_sync_waits(nc, mybir, max_waits=1):
    """walrus in this environment rejects >1 sync wait per instruction;
    hoist excess waits onto dedicated same-engine NOPs."""
    uid = [0]
    for f in nc.m.functions:
        for bb in f.blocks:
            new = []
            dirty = False
            for inst in bb.instructions:
                si = inst.sync_info
                if si is not None and len(si.on_wait) > max_waits:
                    waits = list(si.on_wait)
                    keep = waits[len(waits) - max_waits:]
                    hoist = waits[: len(waits) - max_waits]
                    for i in range(0, len(hoist), max_waits):
                        uid[0] += 1
                        nop = mybir.InstNoOp(
                            name=f"splitwait-{id(nc)}-{uid[0]}", engine=inst.engine
                        )
                        nop.sync_info = mybir.SyncInfo(
                            on_wait=hoist[i : i + max_waits], on_update=[]
                        )
                        nc.register_instruction(nop, overwrite=True)
                        new.append(nop)
                    inst.sync_info = mybir.SyncInfo(
                        on_wait=keep, on_update=list(si.on_update)
                    )
                    dirty = True
                new.append(inst)
            if dirty:
                bb.instructions = new


def _build_program(debug=False, collective=True):
    import concourse.bass as bass
    import concourse.tile as tile
    from concourse import mybir

    f16 = mybir.dt.float16
    f32 = mybir.dt.float32

    nc = bass.Bass("TRN2", target_bir_lowering=False, debug=False, num_devices=8)

    hT = nc.dram_tensor("hT", [DM, L], f16, kind="ExternalInput")
    winxT = nc.dram_tensor("winxT", [DM, CH], f16, kind="ExternalInput")
    winzT = nc.dram_tensor("winzT", [DM, CH], f16, kind="ExternalInput")
    woutT_d = nc.dram_tensor("woutT", [CH, DM], f16, kind="ExternalInput")
    sel_d = nc.dram_tensor("sel", [48, DS * 128], f16, kind="ExternalInput")
    ident_d = nc.dram_tensor("ident", [128, 128], f16, kind="ExternalInput")
    Dg_d = nc.dram_tensor("Dg", [128, NCH * 128], f16, kind="ExternalInput")
    wx_d = {}
    wdt_d = {}
    A_d = {}
    cw_d = {}
    cb_d = {}
    db_d = {}
    D_d = {}
    for d in ("f", "r"):
        wx_d[d] = nc.dram_tensor(f"wx_{d}", [CH, 128], f16, kind="ExternalInput")
        wdt_d[d] = nc.dram_tensor(f"wdt_{d}", [DR, CH], f16, kind="ExternalInput")
        A_d[d] = nc.dram_tensor(f"A_{d}", [128, NCH * DS], f32, kind="ExternalInput")
        cw_d[d] = nc.dram_tensor(f"cw_{d}", [128, NCH * DC], f32, kind="ExternalInput")
        cb_d[d] = nc.dram_tensor(f"cb_{d}", [128, NCH], f32, kind="ExternalInput")
        db_d[d] = nc.dram_tensor(f"db_{d}", [128, NCH], f32, kind="ExternalInput")
        D_d[d] = nc.dram_tensor(f"D_{d}", [128, NCH], f32, kind="ExternalInput")

    pout = nc.dram_tensor("pout", [L, DM], f16, kind="ExternalOutput")
    dbg = {}
    if debug:
        dbg["xc_f"] = nc.dram_tensor("dbg_xc_f", [NCH, 128, L], f16, kind="ExternalOutput")
        dbg["xdbl_f"] = nc.dram_tensor("dbg_xdbl_f", [128, L], f16, kind="ExternalOutput")
        dbg["xdbl_r"] = nc.dram_tensor("dbg_xdbl_r", [128, L], f16, kind="ExternalOutput")
        dbg["dt_f"] = nc.dram_tensor("dbg_dt_f", [NCH, 128, L], f16, kind="ExternalOutput")
        dbg["y_f"] = nc.dram_tensor("dbg_y_f", [NCH, 128, L], f16, kind="ExternalOutput")
        dbg["siluz"] = nc.dram_tensor("dbg_siluz", [NCH, 128, L], f16, kind="ExternalOutput")

    with tile.TileContext(nc, num_cores=8) as tc:
        _build_tile_program(
            nc, tc, tile, mybir, f16, f32,
            hT, winxT, winzT, woutT_d, sel_d, ident_d, Dg_d, wx_d, wdt_d, A_d,
            cw_d, cb_d, db_d, D_d, pout, dbg, collective,
        )

    _split_sync_waits(nc, mybir)
    return nc


def _build_tile_program(
    nc, tc, tile, mybir, f16, f32,
    hT, winxT, winzT, woutT_d, sel_d, ident_d, Dg_d, wx_d, wdt_d, A_d, cw_d,
    cb_d, db_d, D_d, pout, dbg, collective=True,
):
    from contextlib import ExitStack

    AF = mybir.ActivationFunctionType
    OP = mybir.AluOpType
    MM = nc.tensor.matmul
    ACT = nc.scalar.activation
    TT = nc.vector.tensor_tensor
    PTT = nc.gpsimd.tensor_tensor
    STT = nc.vector.scalar_tensor_tensor
    TSMUL = nc.vector.tensor_scalar_mul
    SCAN = nc.vector.tensor_tensor_scan

    ctx = ExitStack()
    with ctx:
        # -------- persistent pools --------
        pers = ctx.enter_context(tc.tile_pool(name="pers", bufs=1))
        dram = ctx.enter_context(tc.tile_pool(name="dram", bufs=1, space="DRAM"))

        sel_sb = pers.tile([48, DS * 128], f16)
        nc.sync.dma_start(sel_sb[:], sel_d[:])
        ident_sb = pers.tile([128, 128], f16)
        nc.sync.dma_start(ident_sb[:], ident_d[:])
        Dg_sb = pers.tile([128, NCH, 128], f16)
        nc.sync.dma_start(Dg_sb[:], Dg_d.ap().rearrange("p (m n) -> p m n", m=NCH))
        carry = {}
        wdt_sb = {}
        A_sb = {}
        db_sb = {}
        D_sb = {}
        for d in ("f", "r"):
            carry[d] = pers.tile([128, NCH, DS], f16, name=f"carry_{d}")
            nc.vector.memset(carry[d][:], 0.0)
            wdt_sb[d] = pers.tile([128, CH], f16, name=f"wdt_sb_{d}")
            nc.sync.dma_start(wdt_sb[d][DR:128, :], wdt_d[d][:])
            A_sb[d] = pers.tile([128, NCH, DS], f32, name=f"A_sb_{d}")
            nc.sync.dma_start(A_sb[d][:], A_d[d].ap().rearrange("p (m s) -> p m s", m=NCH))
            db_sb[d] = pers.tile([128, NCH], f32, name=f"db_sb_{d}")
            nc.sync.dma_start(db_sb[d][:], db_d[d][:])
            D_sb[d] = pers.tile([128, NCH], f32, name=f"D_sb_{d}")
            nc.sync.dma_start(D_sb[d][:], D_d[d][:])
        ones = pers.tile([128, 1], f32)
        nc.vector.memset(ones[:], 1.0)

        # SBUF-resident fp16 intermediates: silu(z) and gated forward output,
        # both in forward time; reverse-direction x_dbl (post-AR, fp16).
        sz_sb = pers.tile([128, NCH, L], f16, name="sz_sb")
        ygf_sb = pers.tile([128, NCH, L], f16, name="ygf_sb")
        xdbl16_r = pers.tile([128, L], f16, name="xdbl16_r")

        # DRAM buffers
        xr_dram = dram.tile([NCH, 128, L], f16)   # reverse-conv out, fwd time
        # chunk-major so each chunk's AllReduce region is contiguous
        ar_in = {d: dram.tile([NC1, 128, T1], f32, name=f"ar_in_{d}") for d in ("f", "r")}
        ar_out = {d: dram.tile([NC1, 128, T1], f32, name=f"ar_out_{d}") for d in ("f", "r")}

        # ---------------- selective-scan chunk (one direction) ------------
        def make_bct_for(d, cc, s, xdbl_ap, p2, p2psum, bct_bufs):
            """B_s|C_s broadcast for chunk cc -> one [128, 2*T2] fp16 tile."""
            bcp = p2psum.tile([128, 2 * T2], f32, tag="bcp", bufs=1,
                              name=f"bcp_{d}_{cc}_{s}")
            MM(bcp[:, 0:T2], sel_sb[0:DS, s * 128 : (s + 1) * 128],
               xdbl_ap(slice(0, DS)), start=True, stop=True)
            MM(bcp[:, T2 : 2 * T2], sel_sb[32 : 32 + DS, s * 128 : (s + 1) * 128],
               xdbl_ap(slice(32, 32 + DS)), start=True, stop=True)
            bct = p2.tile([128, 2 * T2], f16, tag="bct", bufs=bct_bufs,
                          name=f"bct_{d}_{cc}_{s}")
            if d == "f" and (s * 5) % 16 < _BCD:
                nc.vector.tensor_copy(bct[:], bcp[:])
            else:
                ACT(bct[:], bcp[:], AF.Copy)
            return bct

        def scan_chunk(d, c2, p2, p2psum, xdbl_ap, xt_src, ycb,
                       bc_pre=None, prefetch=None, bct_bufs=DS + 1):
            """One chunk of the selective scan for direction d.
            xdbl_ap(rows) -> [rows, T2] fp16 AP of this chunk's reduced x_dbl
            in scan-time order (rows: B 0:16, C 32:48, dt-rank 64:128).
            xt_src(m) -> conv-x AP in scan-time order.  ycb(m, yt, sz_ap):
            consumes the pre-gate output tile per channel-tile.
            bc_pre: pre-produced broadcast tiles (else made JIT in m==0's loop).
            prefetch(m, s): optional hook to interleave next-chunk work."""
            bc_sb = list(bc_pre) if bc_pre is not None else [None] * DS

            def make_bct(s):
                bc_sb[s] = make_bct_for(d, c2, s, xdbl_ap, p2, p2psum, bct_bufs)

            for m in range(NCH):
                psd = p2psum.tile([128, T2], f32, tag="psd", bufs=1,
                                  name=f"psd_{d}_{c2}_{m}")
                MM(psd[:], wdt_sb[d][DR:128, m * 128 : (m + 1) * 128],
                   xdbl_ap(slice(DR, 128)), start=True, stop=True)
                dt = p2.tile([128, T2], f16, tag="dt", bufs=2,
                             name=f"dt_{d}_{c2}_{m}")
                ACT(dt[:], psd[:], AF.Exp, bias=db_sb[d][:, m : m + 1])
                ACT(dt[:], dt[:], AF.Ln, bias=ones[:])
                if dbg and d == "f":
                    nc.sync.dma_start(dbg["dt_f"][m, :, c2 * T2 : (c2 + 1) * T2], dt[:])
                xt_ap = xt_src(m)
                wd = p2.tile([128, T2], f16, tag="wd", bufs=2,
                             name=f"wd_{d}_{c2}_{m}")
                TT(wd[:], dt[:], xt_ap, OP.mult)

                yp = p2psum.tile([128, T2], f32, tag="yp", bufs=2,
                                 name=f"yp_{d}_{c2}_{m}")
                if m == 0 and bc_pre is None:
                    make_bct(0)
                for s in range(DS):
                    dA = p2.tile([128, T2], f16, tag="dA", bufs=4,
                                 name=f"dA_{d}_{c2}_{m}_{s}")
                    ACT(dA[:], dt[:], AF.Exp, scale=A_sb[d][:, m, s : s + 1])
                    # broadcasts are produced one state ahead (after dA so the
                    # scan is never starved), interleaved with the first
                    # channel-tile's state loop so the Act engine never bursts
                    if m == 0 and s + 1 < DS and bc_pre is None:
                        make_bct(s + 1)
                    if prefetch is not None:
                        prefetch(m, s)
                    bt = p2.tile([128, T2], f16, tag="bt", bufs=4,
                                 name=f"bt_{d}_{c2}_{m}_{s}")
                    bt_eng = PTT if ((s * 4 + m) * 5) % 16 < (_PB if d == "f" else _PBR) else TT
                    bt_eng(bt[:], wd[:], bc_sb[s][:, 0:T2], OP.mult)
                    hs = p2.tile([128, T2], f16, tag="hs", bufs=6,
                                 name=f"hs_{d}_{c2}_{m}_{s}")
                    SCAN(hs[:], dA[:], bt[:],
                         carry[d][:, m, s : s + 1], OP.mult, OP.add)
                    nc.vector.tensor_copy(
                        carry[d][:, m, s : s + 1], hs[:, T2 - 1 : T2])
                    cm = p2.tile([128, T2], f16, tag="cm", bufs=4,
                                 name=f"cm_{d}_{c2}_{m}_{s}")
                    cm_eng = PTT if ((s * 4 + m) * 5 + 8) % 16 < (_PC if d == "f" else _PCR) else TT
                    cm_eng(cm[:], hs[:], bc_sb[s][:, T2 : 2 * T2], OP.mult)
                    MM(yp[:], ident_sb[:], cm[:],
                       start=(s == 0), stop=(s == DS - 1 and d != "f"))

                # gating: y = (y + x*D) * silu(z)   (sz read in scan time)
                if d == "f":
                    # x*D folded into the PSUM reduction via block-diag(D)
                    # weights; gate is then a single mixed-dtype TT on PSUM
                    MM(yp[:], Dg_sb[:, m, :], xt_ap, start=False, stop=True)
                    sz_ap = sz_sb[:, m, c2 * T2 : (c2 + 1) * T2]
                    ycb(m, yp, sz_ap)
                else:
                    yt = p2.tile([128, T2], f16, tag="yt", bufs=2,
                                 name=f"yt_{d}_{c2}_{m}")
                    STT(yt[:], xt_ap, D_sb[d][:, m : m + 1], yp[:],
                        OP.mult, OP.add)
                    sz_ap = sz_sb[:, m, L - (c2 + 1) * T2 : L - c2 * T2][:, ::-1]
                    ycb(m, yt, sz_ap)

        # ================= PASS A: in_proj + conv + AR + forward scan ======
        with tc.tile_pool(name="pA", bufs=1) as pA, \
             tc.tile_pool(name="pApsum", bufs=1, space="PSUM") as pApsum:
            winx_sb = pA.tile([128, DM // 128, CH], f16)
            nc.sync.dma_start(winx_sb[:], winxT.ap().rearrange("(k p) n -> p k n", p=128))
            winz_sb = pA.tile([128, DM // 128, CH], f16)
            nc.scalar.dma_start(winz_sb[:], winzT.ap().rearrange("(k p) n -> p k n", p=128))
            wx_sb = {}
            cw_sb = {}
            cb_sb = {}
            for d in ("f", "r"):
                wx_sb[d] = pA.tile([128, NCH, 128], f16, name=f"wx_sb_{d}")
                dqw = nc.sync if d == "f" else nc.scalar
                dqw.dma_start(wx_sb[d][:], wx_d[d].ap().rearrange("(m p) n -> p m n", p=128))
                cw_sb[d] = pA.tile([128, NCH, DC], f32, name=f"cw_sb_{d}")
                nc.sync.dma_start(cw_sb[d][:], cw_d[d].ap().rearrange("p (m j) -> p m j", m=NCH))
                cb_sb[d] = pA.tile([128, NCH], f32, name=f"cb_sb_{d}")
                nc.sync.dma_start(cb_sb[d][:], cb_d[d][:])

            hT_r = hT.ap().rearrange("(k p) l -> p k l", p=128)
            prev_xe = [None] * NCH

            def conv_dir(cc, d, xe_list, out_tiles):
                for m in range(NCH):
                    xc = out_tiles[m]
                    xe = xe_list[m]
                    tmps = []
                    for j in range(DC):
                        off = j if d == "f" else (6 - j)
                        src = xe[:, off : off + T1]
                        wj = cw_sb[d][:, m, j : j + 1]
                        tj = pA.tile([128, T1], f16, tag=f"cvt{j}", bufs=2,
                                     name=f"cvt{j}_{d}_{cc}_{m}")
                        TSMUL(tj[:], src, wj)
                        tmps.append(tj)
                    PTT(tmps[0][:], tmps[0][:], tmps[1][:], OP.add)
                    PTT(tmps[2][:], tmps[2][:], tmps[3][:], OP.add)
                    TT(tmps[0][:], tmps[0][:], tmps[2][:], OP.add)
                    ACT(xc[:], tmps[0][:], AF.Silu, bias=cb_sb[d][:, m : m + 1])

            def xdbl_to_ar(cc, d, xc_tiles):
                """x_dbl partial for this chunk -> fp32 scratch -> ar_in, then
                AllReduce the chunk and convert to fp16."""
                ps = pApsum.tile([128, T1], f32, tag="psx", bufs=1,
                                 name=f"psx_{d}_{cc}")
                for m in range(NCH):
                    MM(ps[:], wx_sb[d][:, m, :], xc_tiles[m][:],
                       start=(m == 0), stop=(m == NCH - 1))
                s32 = pA.tile([128, T1], f32, tag="s32", bufs=2,
                              name=f"s32_{d}_{cc}")
                if d == "f":
                    ACT(s32[:], ps[:], AF.Copy)
                    blk = cc                      # forward-time chunk index
                else:
                    nc.vector.tensor_copy(s32[:], ps[:, ::-1])
                    blk = NC1 - 1 - cc            # flipped-time chunk index
                dq = nc.sync if d == "f" else nc.scalar
                dq.dma_start(ar_in[d][blk], s32[:])
                if collective:
                    nc.gpsimd.collective_compute(
                        "AllReduce", OP.add,
                        replica_groups=[[0, 1, 2, 3], [4, 5, 6, 7]],
                        ins=[ar_in[d][blk].opt()], outs=[ar_out[d][blk].opt()],
                    )
                else:
                    dq.dma_start(ar_out[d][blk], ar_in[d][blk])
                # fp32 -> fp16 conversion fused into the (Pool-engine) DMA load
                if d == "f":
                    x16 = pA.tile([128, T1], f16, tag="x16f", bufs=3,
                                  name=f"x16f_{cc}")
                    nc.gpsimd.dma_start(x16[:], ar_out[d][blk])
                    if dbg:
                        nc.sync.dma_start(
                            dbg["xdbl_f"][:, blk * T1 : (blk + 1) * T1], x16[:])
                    return x16
                sl = slice(blk * T1, (blk + 1) * T1)
                nc.gpsimd.dma_start(xdbl16_r[:, sl], ar_out[d][blk])
                if dbg:
                    nc.sync.dma_start(dbg["xdbl_r"][:, sl], xdbl16_r[:, sl])
                return None

            def finish_reverse(cc, xe_list):
                xcr = [pA.tile([128, T1], f16, tag=f"xcr{m}", bufs=2,
                               name=f"xcr{m}_{cc}") for m in range(NCH)]
                conv_dir(cc, "r", xe_list, xcr)
                for m in range(NCH):
                    dq = nc.sync if m % 2 == 0 else nc.scalar
                    dq.dma_start(xr_dram[m, :, cc * T1 : (cc + 1) * T1], xcr[m][:])
                xdbl_to_ar(cc, "r", xcr)

            for c in range(NC1):
                hTt = pA.tile([128, DM // 128, T1], f16, tag="hTt", bufs=1,
                              name=f"hTt_{c}")
                nk = DM // 128
                nc.sync.dma_start(hTt[:, 0 : nk // 2, :],
                                  hT_r[:, 0 : nk // 2, c * T1 : (c + 1) * T1])
                nc.scalar.dma_start(hTt[:, nk // 2 : nk, :],
                                    hT_r[:, nk // 2 : nk, c * T1 : (c + 1) * T1])

                cur_xe = []
                for m in range(NCH):
                    ps = pApsum.tile([128, T1], f32, tag="ps_ip", bufs=2,
                                     name=f"psx_{c}_{m}")
                    for ko in range(nk):
                        MM(ps[:], winx_sb[:, ko, m * 128 : (m + 1) * 128],
                           hTt[:, ko, :], start=(ko == 0), stop=(ko == nk - 1))
                    xe = pA.tile([128, T1 + 6], f16, tag=f"xe{m}", bufs=2,
                                 name=f"xe{m}_{c}")
                    ACT(xe[:, 3 : 3 + T1], ps[:], AF.Copy)
                    if c == 0:
                        nc.vector.memset(xe[:, 0:3], 0.0)
                    else:
                        nc.vector.tensor_copy(xe[:, 0:3], prev_xe[m][:, T1 : T1 + 3])
                    cur_xe.append(xe)
                for m in range(NCH):
                    ps = pApsum.tile([128, T1], f32, tag="ps_ip", bufs=2,
                                     name=f"psz_{c}_{m}")
                    for ko in range(nk):
                        MM(ps[:], winz_sb[:, ko, m * 128 : (m + 1) * 128],
                           hTt[:, ko, :], start=(ko == 0), stop=(ko == nk - 1))
                    ACT(sz_sb[:, m, c * T1 : (c + 1) * T1], ps[:], AF.Silu)
                    if dbg:
                        nc.sync.dma_start(
                            dbg["siluz"][m, :, c * T1 : (c + 1) * T1],
                            sz_sb[:, m, c * T1 : (c + 1) * T1],
                        )

                if c > 0:
                    for m in range(NCH):
                        nc.vector.tensor_copy(
                            prev_xe[m][:, T1 + 3 : T1 + 6], cur_xe[m][:, 3:6]
                        )
                    finish_reverse(c - 1, prev_xe)

                xcf = [pA.tile([128, T1], f16, tag=f"xcf{m}", bufs=3,
                               name=f"xcf{m}_{c}") for m in range(NCH)]
                conv_dir(c, "f", cur_xe, xcf)
                if dbg:
                    for m in range(NCH):
                        nc.sync.dma_start(
                            dbg["xc_f"][m, :, c * T1 : (c + 1) * T1], xcf[m][:]
                        )
                x16f = xdbl_to_ar(c, "f", xcf)

                # -------- fused forward scan for chunk c --------
                def yg_f(m, yt, sz_ap):
                    TT(ygf_sb[:, m, c * T1 : (c + 1) * T1], yt[:], sz_ap, OP.mult)
                    if dbg:
                        nc.sync.dma_start(
                            dbg["y_f"][m, :, c * T1 : (c + 1) * T1],
                            ygf_sb[:, m, c * T1 : (c + 1) * T1],
                        )

                scan_chunk("f", c, pA, pApsum, lambda rs: x16f[rs, :],
                           lambda m: xcf[m][:], yg_f)

                prev_xe = cur_xe

            for m in range(NCH):
                nc.vector.memset(prev_xe[m][:, T1 + 3 : T1 + 6], 0.0)
            finish_reverse(NC1 - 1, prev_xe)

        # ================= PASS B: reverse scan + combine + out_proj =======
        with tc.tile_pool(name="pB", bufs=1) as pB, \
             tc.tile_pool(name="pBpsum", bufs=1, space="PSUM") as pBpsum:
            wout_sb = pB.tile([128, NCH, DM], f16)
            nc.sync.dma_start(wout_sb[:], woutT_d.ap().rearrange("(k p) n -> p k n", p=128))

            # xdbl16_r is fully available, so each chunk's B/C broadcasts are
            # produced during the PREVIOUS chunk's later channel-tiles, where
            # the Act engine has slack (m==0's loop is otherwise Act-bound).
            BCT_BUFS_B = 2 * DS + 2

            def xdbl_ap_for(cc):
                sl = slice(cc * T2, (cc + 1) * T2)
                return lambda rs: xdbl16_r[rs, sl]

            bct_cur = [
                make_bct_for("r", 0, s, xdbl_ap_for(0), pB, pBpsum, BCT_BUFS_B)
                for s in range(DS)
            ]

            for c2 in range(NC2):
                fsl = slice(L - (c2 + 1) * T2, L - c2 * T2)  # forward window
                xts = []
                bct_nxt = []

                def prefetch(m, s):
                    if c2 + 1 >= NC2 or m == 0:
                        return
                    k = (m - 1) * DS + s
                    if k % 3 == 0 and len(bct_nxt) < DS:
                        bct_nxt.append(make_bct_for(
                            "r", c2 + 1, len(bct_nxt), xdbl_ap_for(c2 + 1),
                            pB, pBpsum, BCT_BUFS_B))

                def xt_src(m):
                    xt = pB.tile([128, T2], f16, tag="xt", bufs=3,
                                 name=f"xt_r_{c2}_{m}")
                    dq = nc.sync if m % 2 == 0 else nc.scalar
                    dq.dma_start(xt[:], xr_dram[m, :, fsl])
                    xts.append(xt)
                    return xt[:, ::-1]

                ytot = []

                def combine(m, yt, sz_ap):
                    # gate in flipped time, then combine into forward time:
                    # ytot[t] = ygf[t] + yg_r[flip t]
                    ygr = pB.tile([128, T2], f16, tag="ygr", bufs=2,
                                  name=f"ygr_{c2}_{m}")
                    TT(ygr[:], yt[:], sz_ap, OP.mult)
                    yt2 = pB.tile([128, T2], f16, tag=f"ytot{m}", bufs=2,
                                  name=f"ytot_{c2}_{m}")
                    TT(yt2[:], ygf_sb[:, m, fsl], ygr[:, ::-1], OP.add)
                    ytot.append(yt2)

                # xdbl16_r is stored in flipped time already
                scan_chunk("r", c2, pB, pBpsum, xdbl_ap_for(c2),
                           xt_src, combine, bc_pre=bct_cur, prefetch=prefetch,
                           bct_bufs=BCT_BUFS_B)
                bct_cur = bct_nxt

                f0 = L - (c2 + 1) * T2
                for mt in range(T2 // 128):
                    ob = pB.tile([128, DM], f16, tag="ob", bufs=2,
                                 name=f"ob_{c2}_{mt}")
                    for nh in range(DM // 512):
                        po = pBpsum.tile([128, 512], f32, tag="po", bufs=2,
                                         name=f"po_{c2}_{mt}_{nh}")
                        for k in range(NCH):
                            MM(po[:], ytot[k][:, mt * 128 : (mt + 1) * 128],
                               wout_sb[:, k, nh * 512 : (nh + 1) * 512],
                               start=(k == 0), stop=(k == NCH - 1))
                        ACT(ob[:, nh * 512 : (nh + 1) * 512], po[:], AF.Copy)
                    dqo = nc.scalar if mt % 2 == 0 else nc.sync
                    dqo.dma_start(
                        pout[f0 + mt * 128 : f0 + (mt + 1) * 128, :], ob[:]
                    )


def _host_prep(inputs):
    """Slice/transpose the full inputs into the 8 per-core input maps."""
    h = np.asarray(inputs["hidden_states"], np.float32)
    W_in = np.asarray(inputs["W_in"], np.float32)
    W_out = np.asarray(inputs["W_out"], np.float32)

    sel = np.zeros((48, DS * 128), np.float16)
    for s in range(DS):
        sel[s, s * 128 : (s + 1) * 128] = 1.0
        sel[32 + s, s * 128 : (s + 1) * 128] = 1.0
    ident = np.eye(128, dtype=np.float16)

    maps = []
    for core in range(8):
        b, g = divmod(core, 4)
        c0 = g * CH
        m = {
            "hT": np.ascontiguousarray(h[b].T).astype(np.float16),
            "winxT": np.ascontiguousarray(W_in[c0 : c0 + CH, :].T).astype(np.float16),
            "winzT": np.ascontiguousarray(W_in[DI + c0 : DI + c0 + CH, :].T).astype(np.float16),
            "woutT": np.ascontiguousarray(W_out[:, c0 : c0 + CH].T).astype(np.float16),
            "sel": sel,
            "ident": ident,
        }
        D_f_full = np.asarray(inputs["D_f"], np.float32)
        Dg = np.zeros((128, NCH * 128), np.float16)
        for mb in range(NCH):
            np.fill_diagonal(
                Dg[:, mb * 128 : (mb + 1) * 128],
                D_f_full[c0 + mb * 128 : c0 + (mb + 1) * 128].astype(np.float16),
            )
        m["Dg"] = Dg
        for d in ("f", "r"):
            sfx = f"_{d}"
            W_x = np.asarray(inputs[f"W_x{sfx}"], np.float32)
            W_dt = np.asarray(inputs[f"W_dt{sfx}"], np.float32)
            A = -np.exp(np.asarray(inputs[f"A_log{sfx}"], np.float64)).astype(np.float32)
            cw = np.asarray(inputs[f"conv_w{sfx}"], np.float32)
            cb = np.asarray(inputs[f"conv_b{sfx}"], np.float32)
            db = np.asarray(inputs[f"b_dt{sfx}"], np.float32)
            Dp = np.asarray(inputs[f"D{sfx}"], np.float32)
            wx_re = np.zeros((CH, 128), np.float16)
            wx_re[:, 0:DS] = W_x[DR : DR + DS, c0 : c0 + CH].T        # B rows
            wx_re[:, 32 : 32 + DS] = W_x[DR + DS : 96, c0 : c0 + CH].T  # C rows
            wx_re[:, DR:128] = W_x[0:DR, c0 : c0 + CH].T              # dt-rank rows
            m[f"wx{sfx}"] = wx_re
            m[f"wdt{sfx}"] = np.ascontiguousarray(W_dt[c0 : c0 + CH, :].T).astype(np.float16)
            m[f"A{sfx}"] = np.ascontiguousarray(
                A[c0 : c0 + CH].reshape(NCH, 128, DS).transpose(1, 0, 2).reshape(128, NCH * DS)
            )
            m[f"cw{sfx}"] = np.ascontiguousarray(
                cw[c0 : c0 + CH].reshape(NCH, 128, DC).transpose(1, 0, 2).reshape(128, NCH * DC)
            )
            m[f"cb{sfx}"] = np.ascontiguousarray(
                cb[c0 : c0 + CH].reshape(NCH, 128).T
            )
            m[f"db{sfx}"] = np.ascontiguousarray(
                db[c0 : c0 + CH].reshape(NCH, 128).T
            )
            m[f"D{sfx}"] = np.ascontiguousarray(
                Dp[c0 : c0 + CH].reshape(NCH, 128).T
            )
        maps.append(m)
    return maps


def run(inputs, debug=False, trace=False):
    from concourse.bass_utils import run_bass_kernel_spmd

    if _COMPILED[0] is None or _COMPILED[0][1] != debug:
        _COMPILED[0] = (_build_program(debug=debug), debug)
    nc = _COMPILED[0][0]
    maps = _host_prep(inputs)
    res = run_bass_kernel_spmd(nc, maps, core_ids=list(range(8)), trace=trace)
    outs = [r["pout"] for r in res.results]
    full = np.zeros((B, L, DM), np.float32)
    for core in range(8):
        b = core // 4
        full[b] += outs[core].astype(np.float32)
    return full, res


def kernel(**inputs):
    out, _ = run(inputs, debug=False, trace=False)
    return out
